# revision 8
# baseline (speedup 1.0000x reference)
"""Two-layer Kipf GCN on 8 Trainium2 NeuronCores (Bass/Tile).

Strategy (sharding_hint): nodes row-sharded across 8 cores (12500 each);
edges partitioned by destination core. Per layer:
  h' = dinv * (x_local @ W)        (PE, per-core rows)  -> AllGather table
  out[d] = dinv[d] * sum_{e: dst=d} h'[src_e]           (edge aggregation)
The aggregation gathers h'[src] rows with dma_gather (int16 indices inside
32k-row source windows), builds a binary one-hot S^T per 128-edge chunk on
DVE (is_equal against an iota tile) and segment-sums via PE matmuls
accumulated in PSUM. The deg^-1/2 norm is folded into per-row scalings
(norm = dinv[src]*dinv[dst]), so S stays binary.

All per-core group sizes are padded to the max across cores so one SPMD
program serves all 8 cores. Pad edges gather row 0 of the window and carry
dst_local=999 which matches no iota column, contributing exactly zero.
"""
import numpy as np
import ml_dtypes

N_NODES = 100000
N_EDGES = 3200000
F_IN, F_HID, N_CLASS = 512, 256, 41
NCORES = 8
R = N_NODES // NCORES          # 12500 rows per core
TILE = 128
T = (R + TILE - 1) // TILE     # 98 dst tiles per core (97 full + 84)
WIN = 32768                    # int16 gather window
W = (N_NODES + WIN - 1) // WIN  # 4 source windows
SUP = 2                        # dst tiles per gather super-group
NSUP = T // SUP                # 49
PAD_DL = 999.0
F2 = 128                       # padded layer-2 feature width

BF16NP = ml_dtypes.bfloat16

assert T % SUP == 0 and NSUP * SUP == T


# ----------------------------------------------------------------------
# Host-side preprocessing
# ----------------------------------------------------------------------
def _preprocess(edge_index):
    src = np.asarray(edge_index[0]).astype(np.int64, copy=False)
    dst = np.asarray(edge_index[1]).astype(np.int64, copy=False)
    loop = np.arange(N_NODES, dtype=np.int64)
    src = np.concatenate([src, loop]).astype(np.int32)
    dst = np.concatenate([dst, loop]).astype(np.int32)

    deg = np.bincount(dst, minlength=N_NODES).astype(np.float32)
    dinv = deg ** -0.5

    # group id in processing order: (super, window, tile_within_super)
    per_core = []
    n_groups = T * W
    sizes_all = np.zeros((NCORES, n_groups), np.int64)
    for m in range(NCORES):
        mask = (dst >= m * R) & (dst < (m + 1) * R)
        es, ed = src[mask], dst[mask]
        tile_id = (ed - m * R) // TILE
        win = es >> 15
        gid = ((tile_id // SUP) * W + win) * SUP + (tile_id % SUP)
        order = np.argsort(gid, kind="stable")
        es, ed, gid = es[order], ed[order], gid[order]
        sizes_all[m] = np.bincount(gid, minlength=n_groups)
        per_core.append((es, ed, gid))

    C = (np.max(sizes_all, axis=0) + TILE - 1) // TILE
    C = np.maximum(C, 1)                      # uniform non-empty structure
    L = C * TILE
    goff = np.zeros(n_groups + 1, np.int64)
    goff[1:] = np.cumsum(L)
    coff = np.zeros(n_groups + 1, np.int64)
    coff[1:] = np.cumsum(C)
    total_e = int(goff[-1])
    total_c = int(coff[-1])

    cores = []
    for m in range(NCORES):
        es, ed, gid = per_core[m]
        sizes = sizes_all[m]
        gstart = np.zeros(n_groups, np.int64)
        gstart[1:] = np.cumsum(sizes)[:-1]
        local = np.arange(len(es)) - gstart[gid]
        pos = goff[gid] + local

        idx16 = np.zeros(total_e, np.int16)
        dl = np.full(total_e, PAD_DL, np.float32)
        idx16[pos] = (es - (es >> 15) * WIN).astype(np.int16)
        tile_of_gid = (gid // (W * SUP)) * SUP + (gid % SUP)
        dl[pos] = (ed - m * R - tile_of_gid * TILE).astype(np.float32)

        idx_packed = np.ascontiguousarray(
            np.tile(idx16.reshape(-1, 16).T, (8, 1))
        )  # [128, total_e//16] int16
        dl_cols = np.ascontiguousarray(
            dl.reshape(total_c, TILE).T
        )  # [128, total_c] f32

        dinv_col = np.ones((TILE, T), np.float32)
        dm = dinv[m * R : (m + 1) * R]
        full, tail = divmod(R, TILE)
        dinv_col[:, :full] = dm[: full * TILE].reshape(full, TILE).T
        if tail:
            dinv_col[:tail, full] = dm[full * TILE:]

        cores.append(dict(idx=idx_packed, dl=dl_cols, dinv_col=dinv_col))

    meta = dict(C=C, goff=goff, coff=coff, total_e=total_e, total_c=total_c)
    return cores, meta


# ----------------------------------------------------------------------
# Device program
# ----------------------------------------------------------------------
def _build_program(meta):
    import concourse.bacc as bacc
    import concourse.tile as tile
    import concourse.mybir as mybir
    from contextlib import ExitStack

    F32 = mybir.dt.float32
    BF16 = mybir.dt.bfloat16
    I16 = mybir.dt.int16
    EQ = mybir.AluOpType.is_equal
    MUL = mybir.AluOpType.mult
    SUB = mybir.AluOpType.subtract
    ADD = mybir.AluOpType.add
    AX = mybir.AxisListType.X
    ACT = mybir.ActivationFunctionType

    C = meta["C"]
    goff = meta["goff"]
    coff = meta["coff"]
    total_e = meta["total_e"]
    total_c = meta["total_c"]
    n_groups = T * W

    def gid_of(s, w, tl):
        return (s * W + w) * SUP + tl

    nc = bacc.Bacc("TRN2", target_bir_lowering=False, debug=False,
                   num_devices=NCORES)

    xT_t = nc.declare_dram_parameter("xT", [F_IN, R], BF16, isOutput=False)
    w1_t = nc.declare_dram_parameter("w1", [F_IN, F_HID], BF16, isOutput=False)
    w2_t = nc.declare_dram_parameter("w2", [F_HID, F2], BF16, isOutput=False)
    b1_t = nc.declare_dram_parameter("b1rep", [128, F_HID], F32, isOutput=False)
    b2_t = nc.declare_dram_parameter("b2rep", [128, F2], F32, isOutput=False)
    iota_t = nc.declare_dram_parameter("iota", [128, 128], BF16, isOutput=False)
    ident_t = nc.declare_dram_parameter("ident", [128, 128], BF16, isOutput=False)
    dinv_t = nc.declare_dram_parameter("dinv_col", [128, T], F32, isOutput=False)
    idx_t = nc.declare_dram_parameter("idx", [128, total_e // 16], I16,
                                      isOutput=False)
    dl_t = nc.declare_dram_parameter("dl", [128, total_c], F32, isOutput=False)

    out_t = nc.declare_dram_parameter("out", [R, N_CLASS], F32, isOutput=True)

    ag1_in = nc.dram_tensor("ag1_in", [R, F_HID], BF16)
    h1full = nc.dram_tensor("h1full", [N_NODES, F_HID], BF16, addr_space="Shared")
    ag2_in = nc.dram_tensor("ag2_in", [R, F2], BF16)
    h2full = nc.dram_tensor("h2full", [N_NODES, F2], BF16, addr_space="Shared")

    rg = [list(range(NCORES))]

    with tile.TileContext(nc) as tc, ExitStack() as ctx:
        const = ctx.enter_context(tc.tile_pool(name="const", bufs=1))
        psum = ctx.enter_context(tc.tile_pool(name="psum", bufs=2, space="PSUM"))

        # ---- constants ----
        w1_s = const.tile([128, F_IN // 128, F_HID], BF16)
        nc.sync.dma_start(
            w1_s[:], w1_t[:].rearrange("(a p) f -> p a f", p=128)
        )
        w2_s = const.tile([128, F_HID // 128, F2], BF16)
        nc.sync.dma_start(
            w2_s[:], w2_t[:].rearrange("(a p) f -> p a f", p=128)
        )
        b1_s = const.tile([128, F_HID], F32)
        nc.sync.dma_start(b1_s[:], b1_t[:])
        b2_s = const.tile([128, F2], F32)
        nc.sync.dma_start(b2_s[:], b2_t[:])
        iota_s = const.tile([128, 128], BF16)
        nc.sync.dma_start(iota_s[:], iota_t[:])
        ident_s = const.tile([128, 128], BF16)
        nc.sync.dma_start(ident_s[:], ident_t[:])
        dinv_s = const.tile([128, T], F32)
        nc.sync.dma_start(dinv_s[:], dinv_t[:])
        idx_s = const.tile([128, total_e // 16], I16)
        nc.sync.dma_start(idx_s[:], idx_t[:])
        dl_s = const.tile([128, total_c], F32)
        nc.sync.dma_start(dl_s[:], dl_t[:])

        # ---- phase 1: h' = dinv * (x @ W1) ----
        with tc.tile_pool(name="xt", bufs=1) as xtp, \
                tc.tile_pool(name="hp", bufs=3) as hpp:
            xT_s = xtp.tile([128, F_IN // 128, R], BF16)
            nc.sync.dma_start(
                xT_s[:], xT_t[:].rearrange("(a p) r -> p a r", p=128)
            )
            for t in range(T):
                mt = min(TILE, R - t * TILE)
                ps = psum.tile([128, F_HID], F32, tag="agg")
                for k in range(F_IN // 128):
                    nc.tensor.matmul(
                        ps[:mt, :],
                        xT_s[:, k, t * TILE : t * TILE + mt],
                        w1_s[:, k, :],
                        start=(k == 0),
                        stop=(k == F_IN // 128 - 1),
                    )
                hp = hpp.tile([128, F_HID], BF16, tag="hp")
                nc.vector.tensor_scalar(
                    hp[:mt, :], ps[:mt, :], dinv_s[:mt, t : t + 1], None, op0=MUL
                )
                nc.sync.dma_start(
                    ag1_in[t * TILE : t * TILE + mt, :], hp[:mt, :]
                )

        vags = ctx.enter_context(tc.tile_pool(name="vags", bufs=2))
        spool = ctx.enter_context(tc.tile_pool(name="spool", bufs=6))
        epool = ctx.enter_context(tc.tile_pool(name="epool", bufs=3))

        nc.gpsimd.collective_compute(
            "AllGather", mybir.AluOpType.bypass, replica_groups=rg,
            ins=[ag1_in[:]], outs=[h1full[:]],
        )

        # ---- aggregation layer (shared for L1/L2) ----
        def agg_layer(layer, table, felem):
            for s in range(NSUP):
                vt = {}
                for w in range(W):
                    g0 = gid_of(s, w, 0)
                    e0, e1 = goff[g0], goff[g0 + SUP]
                    n = int(e1 - e0)
                    wrows = min(WIN, N_NODES - w * WIN)
                    v = vags.tile(
                        [128, (n // 128) * felem], BF16, tag=f"v_{w}"
                    )
                    nc.gpsimd.dma_gather(
                        v[:].rearrange("p (c f) -> p c f", f=felem),
                        table[w * WIN : w * WIN + wrows, :],
                        idx_s[:, int(e0) // 16 : int(e1) // 16],
                        n, n, felem, single_packet=False,
                    )
                    vt[w] = v
                for tl in range(SUP):
                    t = s * SUP + tl
                    ps = psum.tile([128, felem], F32, tag="agg")
                    ngroups = [(w, gid_of(s, w, tl)) for w in range(W)]
                    nchunks = sum(int(C[g]) for _, g in ngroups)
                    ci = 0
                    for w, g in ngroups:
                        cbase = coff[g] - coff[gid_of(s, w, 0)]
                        for c in range(int(C[g])):
                            st = spool.tile([128, 128], BF16, tag="s")
                            col = int(coff[g]) + c
                            nc.vector.tensor_scalar(
                                st[:], iota_s[:], dl_s[:, col : col + 1],
                                None, op0=EQ,
                            )
                            vcol = (int(cbase) + c) * felem
                            nc.tensor.matmul(
                                ps[:],
                                st[:],
                                vt[w][:, vcol : vcol + felem],
                                start=(ci == 0),
                                stop=(ci == nchunks - 1),
                            )
                            ci += 1
                    yield t, ps

        # ---- L1 aggregation + relu + W2 ----
        for t, ps in agg_layer(1, h1full, F_HID):
            mt = min(TILE, R - t * TILE)
            h1 = epool.tile([128, F_HID], F32, tag="h1")
            nc.vector.tensor_scalar(
                h1[:], ps[:], dinv_s[:, t : t + 1], None, op0=MUL
            )
            h1b = epool.tile([128, F_HID], F32, tag="h1b")
            nc.vector.tensor_tensor(h1b[:], h1[:], b1_s[:], op=ADD)
            rt = epool.tile([128, F_HID], BF16, tag="rt")
            nc.scalar.activation(rt[:], h1b[:], ACT.Relu)
            zp = psum.tile([128, F2], F32, tag="z")
            for k in range(F_HID // 128):
                tp = psum.tile([128, 128], BF16, tag="tp")
                nc.tensor.transpose(
                    tp[:], rt[:, k * 128 : (k + 1) * 128], ident_s[:]
                )
                rT = epool.tile([128, 128], BF16, tag="rT")
                nc.vector.tensor_copy(rT[:], tp[:])
                nc.tensor.matmul(
                    zp[:], rT[:], w2_s[:, k, :],
                    start=(k == 0), stop=(k == F_HID // 128 - 1),
                )
            zt = epool.tile([128, F2], BF16, tag="zt")
            nc.vector.tensor_scalar(
                zt[:], zp[:], dinv_s[:, t : t + 1], None, op0=MUL
            )
            nc.sync.dma_start(ag2_in[t * TILE : t * TILE + mt, :], zt[:mt, :])

        nc.gpsimd.collective_compute(
            "AllGather", mybir.AluOpType.bypass, replica_groups=rg,
            ins=[ag2_in[:]], outs=[h2full[:]],
        )

        # ---- L2 aggregation + bias + log_softmax ----
        for t, ps in agg_layer(2, h2full, F2):
            mt = min(TILE, R - t * TILE)
            lt = epool.tile([128, N_CLASS], F32, tag="lt")
            nc.vector.tensor_scalar(
                lt[:], ps[:, :N_CLASS], dinv_s[:, t : t + 1], None, op0=MUL
            )
            lt2 = epool.tile([128, N_CLASS], F32, tag="lt2")
            nc.vector.tensor_tensor(lt2[:], lt[:], b2_s[:, :N_CLASS], op=ADD)
            nmx = epool.tile([128, 1], F32, tag="nmx")
            nc.vector.reduce_max(nmx[:], lt2[:], axis=AX, negate=True)
            ex = epool.tile([128, N_CLASS], F32, tag="ex")
            sm = epool.tile([128, 1], F32, tag="sm")
            nc.scalar.activation(
                ex[:], lt2[:], ACT.Exp, bias=nmx[:], accum_out=sm[:]
            )
            lg = epool.tile([128, 1], F32, tag="lg")
            nc.scalar.activation(lg[:], sm[:], ACT.Ln)
            lse = epool.tile([128, 1], F32, tag="lse")
            nc.vector.tensor_tensor(lse[:], lg[:], nmx[:], op=SUB)
            of = epool.tile([128, N_CLASS], F32, tag="of")
            nc.vector.tensor_scalar(
                of[:], lt2[:], lse[:], None, op0=SUB
            )
            nc.sync.dma_start(out_t[t * TILE : t * TILE + mt, :], of[:mt, :])

    nc.compile()
    return nc


_CACHE = {}


def _get_program_and_meta(edge_index):
    key = "prog"
    if key not in _CACHE:
        cores, meta = _preprocess(edge_index)
        nc = _build_program(meta)
        _CACHE[key] = (nc, cores, meta)
    return _CACHE[key]


def kernel(x, edge_index, W1, b1, W2, b2, _trace=False):
    from concourse.bass_utils import run_bass_kernel_spmd

    nc, cores, meta = _get_program_and_meta(edge_index)

    x = np.asarray(x, dtype=np.float32)
    W1b = np.asarray(W1, dtype=np.float32).astype(BF16NP)
    W2p = np.zeros((F_HID, F2), np.float32)
    W2p[:, :N_CLASS] = np.asarray(W2, dtype=np.float32)
    W2pb = W2p.astype(BF16NP)
    b1rep = np.tile(np.asarray(b1, np.float32)[None, :], (128, 1))
    b2p = np.zeros(F2, np.float32)
    b2p[:N_CLASS] = np.asarray(b2, np.float32)
    b2rep = np.tile(b2p[None, :], (128, 1))
    iota = np.tile(
        np.arange(128, dtype=np.float32)[None, :], (128, 1)
    ).astype(BF16NP)
    ident = np.eye(128, dtype=np.float32).astype(BF16NP)

    xb = x.astype(BF16NP)
    in_maps = []
    for m in range(NCORES):
        xT_m = np.ascontiguousarray(xb[m * R : (m + 1) * R].T)
        in_maps.append(
            dict(
                xT=xT_m, w1=W1b, w2=W2pb, b1rep=b1rep, b2rep=b2rep,
                iota=iota, ident=ident, dinv_col=cores[m]["dinv_col"],
                idx=cores[m]["idx"], dl=cores[m]["dl"],
            )
        )

    res = run_bass_kernel_spmd(
        nc, in_maps, list(range(NCORES)), trace=_trace,
        trace_cores=[0] if _trace else None,
    )
    out = np.concatenate([res.results[m]["out"] for m in range(NCORES)], axis=0)
    if _trace:
        kernel.last_exec_time_ns = res.exec_time_ns
    return out


# revision 9
# speedup vs baseline: 1.2706x; 1.2706x over previous
"""Two-layer Kipf GCN on 8 Trainium2 NeuronCores (Bass/Tile).

Strategy (sharding_hint): nodes row-sharded across 8 cores (12500 each);
edges partitioned by destination core. Per layer:
  h' = dinv * (x_local @ W)        (PE, per-core rows)  -> AllGather table
  out[d] = dinv[d] * sum_{e: dst=d} h'[src_e]           (edge aggregation)
The aggregation gathers h'[src] rows with dma_gather (int16 indices inside
32k-row source windows), builds a binary one-hot S^T per 128-edge chunk on
DVE (is_equal against an iota tile) and segment-sums via PE matmuls
accumulated in PSUM. The deg^-1/2 norm is folded into per-row scalings
(norm = dinv[src]*dinv[dst]), so S stays binary.

All per-core group sizes are padded to the max across cores so one SPMD
program serves all 8 cores. Pad edges gather row 0 of the window and carry
dst_local=999 which matches no iota column, contributing exactly zero.
"""
import numpy as np
import ml_dtypes

N_NODES = 100000
N_EDGES = 3200000
F_IN, F_HID, N_CLASS = 512, 256, 41
NCORES = 8
R = N_NODES // NCORES          # 12500 rows per core
TILE = 128
T = (R + TILE - 1) // TILE     # 98 dst tiles per core (97 full + 84)
WIN = 32768                    # int16 gather window
W = (N_NODES + WIN - 1) // WIN  # 4 source windows
SUP = 2                        # dst tiles per gather super-group
NSUP = T // SUP                # 49
PAD_DL = 999.0
F2 = 128                       # padded layer-2 feature width

BF16NP = ml_dtypes.bfloat16

assert T % SUP == 0 and NSUP * SUP == T


# ----------------------------------------------------------------------
# Host-side preprocessing
# ----------------------------------------------------------------------
def _preprocess(edge_index):
    src = np.asarray(edge_index[0]).astype(np.int64, copy=False)
    dst = np.asarray(edge_index[1]).astype(np.int64, copy=False)
    loop = np.arange(N_NODES, dtype=np.int64)
    src = np.concatenate([src, loop]).astype(np.int32)
    dst = np.concatenate([dst, loop]).astype(np.int32)

    deg = np.bincount(dst, minlength=N_NODES).astype(np.float32)
    dinv = deg ** -0.5

    # group id in processing order: (super, window, tile_within_super)
    per_core = []
    n_groups = T * W
    sizes_all = np.zeros((NCORES, n_groups), np.int64)
    for m in range(NCORES):
        mask = (dst >= m * R) & (dst < (m + 1) * R)
        es, ed = src[mask], dst[mask]
        tile_id = (ed - m * R) // TILE
        win = es >> 15
        gid = ((tile_id // SUP) * W + win) * SUP + (tile_id % SUP)
        order = np.argsort(gid, kind="stable")
        es, ed, gid = es[order], ed[order], gid[order]
        sizes_all[m] = np.bincount(gid, minlength=n_groups)
        per_core.append((es, ed, gid))

    C = (np.max(sizes_all, axis=0) + TILE - 1) // TILE
    C = np.maximum(C, 1)                      # uniform non-empty structure
    L = C * TILE
    goff = np.zeros(n_groups + 1, np.int64)
    goff[1:] = np.cumsum(L)
    coff = np.zeros(n_groups + 1, np.int64)
    coff[1:] = np.cumsum(C)
    total_e = int(goff[-1])
    total_c = int(coff[-1])

    cores = []
    for m in range(NCORES):
        es, ed, gid = per_core[m]
        sizes = sizes_all[m]
        gstart = np.zeros(n_groups, np.int64)
        gstart[1:] = np.cumsum(sizes)[:-1]
        local = np.arange(len(es)) - gstart[gid]
        pos = goff[gid] + local

        idx16 = np.zeros(total_e, np.int16)
        dl = np.full(total_e, PAD_DL, np.float32)
        idx16[pos] = (es - (es >> 15) * WIN).astype(np.int16)
        tile_of_gid = (gid // (W * SUP)) * SUP + (gid % SUP)
        dl[pos] = (ed - m * R - tile_of_gid * TILE).astype(np.float32)

        idx_packed = np.ascontiguousarray(
            np.tile(idx16.reshape(-1, 16).T, (8, 1))
        )  # [128, total_e//16] int16
        dl_cols = np.ascontiguousarray(
            dl.reshape(total_c, TILE).T
        )  # [128, total_c] f32

        dinv_col = np.ones((TILE, T), np.float32)
        dm = dinv[m * R : (m + 1) * R]
        full, tail = divmod(R, TILE)
        dinv_col[:, :full] = dm[: full * TILE].reshape(full, TILE).T
        if tail:
            dinv_col[:tail, full] = dm[full * TILE:]

        cores.append(dict(idx=idx_packed, dl=dl_cols, dinv_col=dinv_col))

    meta = dict(C=C, goff=goff, coff=coff, total_e=total_e, total_c=total_c)
    return cores, meta


# ----------------------------------------------------------------------
# Device program
# ----------------------------------------------------------------------
def _build_program(meta):
    import concourse.bacc as bacc
    import concourse.tile as tile
    import concourse.mybir as mybir
    from contextlib import ExitStack

    F32 = mybir.dt.float32
    BF16 = mybir.dt.bfloat16
    I16 = mybir.dt.int16
    EQ = mybir.AluOpType.is_equal
    MUL = mybir.AluOpType.mult
    SUB = mybir.AluOpType.subtract
    ADD = mybir.AluOpType.add
    AX = mybir.AxisListType.X
    ACT = mybir.ActivationFunctionType

    C = meta["C"]
    goff = meta["goff"]
    coff = meta["coff"]
    total_e = meta["total_e"]
    total_c = meta["total_c"]
    n_groups = T * W

    def gid_of(s, w, tl):
        return (s * W + w) * SUP + tl

    nc = bacc.Bacc("TRN2", target_bir_lowering=False, debug=False,
                   num_devices=NCORES, num_swdge_queues=4)

    xT_t = nc.declare_dram_parameter("xT", [F_IN, R], BF16, isOutput=False)
    w1_t = nc.declare_dram_parameter("w1", [F_IN, F_HID], BF16, isOutput=False)
    w2_t = nc.declare_dram_parameter("w2", [F_HID, F2], BF16, isOutput=False)
    b1_t = nc.declare_dram_parameter("b1rep", [128, F_HID], F32, isOutput=False)
    b2_t = nc.declare_dram_parameter("b2rep", [128, F2], F32, isOutput=False)
    iota_t = nc.declare_dram_parameter("iota", [128, 128], BF16, isOutput=False)
    ident_t = nc.declare_dram_parameter("ident", [128, 128], BF16, isOutput=False)
    dinv_t = nc.declare_dram_parameter("dinv_col", [128, T], F32, isOutput=False)
    idx_t = nc.declare_dram_parameter("idx", [128, total_e // 16], I16,
                                      isOutput=False)
    dl_t = nc.declare_dram_parameter("dl", [128, total_c], F32, isOutput=False)

    out_t = nc.declare_dram_parameter("out", [R, N_CLASS], F32, isOutput=True)

    ag1_in = nc.dram_tensor("ag1_in", [R, F_HID], BF16)
    h1full = nc.dram_tensor("h1full", [N_NODES, F_HID], BF16, addr_space="Shared")
    ag2_in = nc.dram_tensor("ag2_in", [R, F2], BF16)
    h2full = nc.dram_tensor("h2full", [N_NODES, F2], BF16, addr_space="Shared")

    rg = [list(range(NCORES))]

    with tile.TileContext(nc) as tc, ExitStack() as ctx:
        const = ctx.enter_context(tc.tile_pool(name="const", bufs=1))
        psum = ctx.enter_context(tc.tile_pool(name="psum", bufs=2, space="PSUM"))

        # ---- constants ----
        w1_s = const.tile([128, F_IN // 128, F_HID], BF16)
        nc.sync.dma_start(
            w1_s[:], w1_t[:].rearrange("(a p) f -> p a f", p=128)
        )
        w2_s = const.tile([128, F_HID // 128, F2], BF16)
        nc.sync.dma_start(
            w2_s[:], w2_t[:].rearrange("(a p) f -> p a f", p=128)
        )
        b1_s = const.tile([128, F_HID], F32)
        nc.sync.dma_start(b1_s[:], b1_t[:])
        b2_s = const.tile([128, F2], F32)
        nc.sync.dma_start(b2_s[:], b2_t[:])
        iota_s = const.tile([128, 128], BF16)
        nc.sync.dma_start(iota_s[:], iota_t[:])
        ident_s = const.tile([128, 128], BF16)
        nc.sync.dma_start(ident_s[:], ident_t[:])
        dinv_s = const.tile([128, T], F32)
        nc.sync.dma_start(dinv_s[:], dinv_t[:])
        idx_s = const.tile([128, total_e // 16], I16)
        nc.sync.dma_start(idx_s[:], idx_t[:])
        dl_s = const.tile([128, total_c], F32)
        nc.sync.dma_start(dl_s[:], dl_t[:])

        # ---- phase 1: h' = dinv * (x @ W1) ----
        with tc.tile_pool(name="xt", bufs=1) as xtp, \
                tc.tile_pool(name="hp", bufs=3) as hpp:
            xT_s = xtp.tile([128, F_IN // 128, R], BF16)
            nc.sync.dma_start(
                xT_s[:], xT_t[:].rearrange("(a p) r -> p a r", p=128)
            )
            for t in range(T):
                mt = min(TILE, R - t * TILE)
                ps = psum.tile([128, F_HID], F32, tag="agg")
                for k in range(F_IN // 128):
                    nc.tensor.matmul(
                        ps[:mt, :],
                        xT_s[:, k, t * TILE : t * TILE + mt],
                        w1_s[:, k, :],
                        start=(k == 0),
                        stop=(k == F_IN // 128 - 1),
                    )
                hp = hpp.tile([128, F_HID], BF16, tag="hp")
                nc.vector.tensor_scalar(
                    hp[:mt, :], ps[:mt, :], dinv_s[:mt, t : t + 1], None, op0=MUL
                )
                nc.sync.dma_start(
                    ag1_in[t * TILE : t * TILE + mt, :], hp[:mt, :]
                )

        vags = ctx.enter_context(tc.tile_pool(name="vags", bufs=2))
        spool = ctx.enter_context(tc.tile_pool(name="spool", bufs=6))
        epool = ctx.enter_context(tc.tile_pool(name="epool", bufs=3))

        nc.gpsimd.collective_compute(
            "AllGather", mybir.AluOpType.bypass, replica_groups=rg,
            ins=[ag1_in[:]], outs=[h1full[:]],
        )

        # ---- aggregation layer (shared for L1/L2) ----
        def agg_layer(layer, table, felem):
            for s in range(NSUP):
                vt = {}
                for w in range(W):
                    g0 = gid_of(s, w, 0)
                    e0, e1 = goff[g0], goff[g0 + SUP]
                    n = int(e1 - e0)
                    wrows = min(WIN, N_NODES - w * WIN)
                    v = vags.tile(
                        [128, (n // 128) * felem], BF16, tag=f"v_{w}"
                    )
                    nc.gpsimd.dma_gather(
                        v[:].rearrange("p (c f) -> p c f", f=felem),
                        table[w * WIN : w * WIN + wrows, :],
                        idx_s[:, int(e0) // 16 : int(e1) // 16],
                        n, n, felem, single_packet=False, queue_num=w,
                    )
                    vt[w] = v
                for tl in range(SUP):
                    t = s * SUP + tl
                    ps = psum.tile([128, felem], F32, tag="agg")
                    ngroups = [(w, gid_of(s, w, tl)) for w in range(W)]
                    nchunks = sum(int(C[g]) for _, g in ngroups)
                    ci = 0
                    for w, g in ngroups:
                        cbase = coff[g] - coff[gid_of(s, w, 0)]
                        for c in range(int(C[g])):
                            st = spool.tile([128, 128], BF16, tag="s")
                            col = int(coff[g]) + c
                            nc.vector.tensor_scalar(
                                st[:], iota_s[:], dl_s[:, col : col + 1],
                                None, op0=EQ,
                            )
                            vcol = (int(cbase) + c) * felem
                            nc.tensor.matmul(
                                ps[:],
                                st[:],
                                vt[w][:, vcol : vcol + felem],
                                start=(ci == 0),
                                stop=(ci == nchunks - 1),
                            )
                            ci += 1
                    yield t, ps

        # ---- L1 aggregation + relu + W2 ----
        for t, ps in agg_layer(1, h1full, F_HID):
            mt = min(TILE, R - t * TILE)
            h1 = epool.tile([128, F_HID], F32, tag="h1")
            nc.vector.tensor_scalar(
                h1[:], ps[:], dinv_s[:, t : t + 1], None, op0=MUL
            )
            h1b = epool.tile([128, F_HID], F32, tag="h1b")
            nc.vector.tensor_tensor(h1b[:], h1[:], b1_s[:], op=ADD)
            rt = epool.tile([128, F_HID], BF16, tag="rt")
            nc.scalar.activation(rt[:], h1b[:], ACT.Relu)
            zp = psum.tile([128, F2], F32, tag="z")
            for k in range(F_HID // 128):
                tp = psum.tile([128, 128], BF16, tag="tp")
                nc.tensor.transpose(
                    tp[:], rt[:, k * 128 : (k + 1) * 128], ident_s[:]
                )
                rT = epool.tile([128, 128], BF16, tag="rT")
                nc.vector.tensor_copy(rT[:], tp[:])
                nc.tensor.matmul(
                    zp[:], rT[:], w2_s[:, k, :],
                    start=(k == 0), stop=(k == F_HID // 128 - 1),
                )
            zt = epool.tile([128, F2], BF16, tag="zt")
            nc.vector.tensor_scalar(
                zt[:], zp[:], dinv_s[:, t : t + 1], None, op0=MUL
            )
            nc.sync.dma_start(ag2_in[t * TILE : t * TILE + mt, :], zt[:mt, :])

        nc.gpsimd.collective_compute(
            "AllGather", mybir.AluOpType.bypass, replica_groups=rg,
            ins=[ag2_in[:]], outs=[h2full[:]],
        )

        # ---- L2 aggregation + bias + log_softmax ----
        for t, ps in agg_layer(2, h2full, F2):
            mt = min(TILE, R - t * TILE)
            lt = epool.tile([128, N_CLASS], F32, tag="lt")
            nc.vector.tensor_scalar(
                lt[:], ps[:, :N_CLASS], dinv_s[:, t : t + 1], None, op0=MUL
            )
            lt2 = epool.tile([128, N_CLASS], F32, tag="lt2")
            nc.vector.tensor_tensor(lt2[:], lt[:], b2_s[:, :N_CLASS], op=ADD)
            nmx = epool.tile([128, 1], F32, tag="nmx")
            nc.vector.reduce_max(nmx[:], lt2[:], axis=AX, negate=True)
            ex = epool.tile([128, N_CLASS], F32, tag="ex")
            sm = epool.tile([128, 1], F32, tag="sm")
            nc.scalar.activation(
                ex[:], lt2[:], ACT.Exp, bias=nmx[:], accum_out=sm[:]
            )
            lg = epool.tile([128, 1], F32, tag="lg")
            nc.scalar.activation(lg[:], sm[:], ACT.Ln)
            lse = epool.tile([128, 1], F32, tag="lse")
            nc.vector.tensor_tensor(lse[:], lg[:], nmx[:], op=SUB)
            of = epool.tile([128, N_CLASS], F32, tag="of")
            nc.vector.tensor_scalar(
                of[:], lt2[:], lse[:], None, op0=SUB
            )
            nc.sync.dma_start(out_t[t * TILE : t * TILE + mt, :], of[:mt, :])

    nc.compile()
    return nc


_CACHE = {}


def _get_program_and_meta(edge_index):
    key = "prog"
    if key not in _CACHE:
        cores, meta = _preprocess(edge_index)
        nc = _build_program(meta)
        _CACHE[key] = (nc, cores, meta)
    return _CACHE[key]


def kernel(x, edge_index, W1, b1, W2, b2, _trace=False):
    from concourse.bass_utils import run_bass_kernel_spmd

    nc, cores, meta = _get_program_and_meta(edge_index)

    x = np.asarray(x, dtype=np.float32)
    W1b = np.asarray(W1, dtype=np.float32).astype(BF16NP)
    W2p = np.zeros((F_HID, F2), np.float32)
    W2p[:, :N_CLASS] = np.asarray(W2, dtype=np.float32)
    W2pb = W2p.astype(BF16NP)
    b1rep = np.tile(np.asarray(b1, np.float32)[None, :], (128, 1))
    b2p = np.zeros(F2, np.float32)
    b2p[:N_CLASS] = np.asarray(b2, np.float32)
    b2rep = np.tile(b2p[None, :], (128, 1))
    iota = np.tile(
        np.arange(128, dtype=np.float32)[None, :], (128, 1)
    ).astype(BF16NP)
    ident = np.eye(128, dtype=np.float32).astype(BF16NP)

    xb = x.astype(BF16NP)
    in_maps = []
    for m in range(NCORES):
        xT_m = np.ascontiguousarray(xb[m * R : (m + 1) * R].T)
        in_maps.append(
            dict(
                xT=xT_m, w1=W1b, w2=W2pb, b1rep=b1rep, b2rep=b2rep,
                iota=iota, ident=ident, dinv_col=cores[m]["dinv_col"],
                idx=cores[m]["idx"], dl=cores[m]["dl"],
            )
        )

    res = run_bass_kernel_spmd(
        nc, in_maps, list(range(NCORES)), trace=_trace,
        trace_cores=[0] if _trace else None,
    )
    out = np.concatenate([res.results[m]["out"] for m in range(NCORES)], axis=0)
    if _trace:
        kernel.last_exec_time_ns = res.exec_time_ns
    return out
